# revision 1
# baseline (speedup 1.0000x reference)
"""Multi-head attention (B=2, T=4096, D=768, H=12) as a Bass/Tile kernel
for 8 Trainium2 NeuronCores.

Sharding: cores 0-3 own batch 0, cores 4-7 own batch 1; each core owns 3
heads (tensor-parallel over heads, data-parallel over batch). Each core
computes x@Wq/Wk/Wv for its head slice, attention, and its heads' partial
O-projection; the host sums the 4 per-batch partials (the head dimension
of attn @ W_o is a pure reduction). b_o is folded in on one core per
batch via the sumexp trick below.

Per-core pipeline (all matmuls fp32r = full PE rate, fp32 storage):
  A) Host supplies x^T; Q^T/K^T [64, T] come from matmuls with W as the
     stationary operand and x^T as moving. V is produced in natural
     [t, dk] layout (x^T blocks stationary) and written into V_aug
     [128, 65] chunks whose 65th column is 1.0.
  B) scores^T[k, q] = K^T-chunkT @ Q^T — both operands are natural
     slices, no transposes anywhere. Two heads run concurrently on the
     PE via tile_position row packing (rows 0-63 / 64-127); the odd head
     is self-packed across two query chunks (its Q/K live duplicated in
     both partition halves). exp(scores/8) runs on ScalarE straight out
     of PSUM into SBUF (scale fused, no max-subtraction: |scores| < ~3
     so fp32 exp is safe). attn_aug^T[65, q] accumulates
     V_aug-chunk.T @ exp over all 32 key chunks; row 64 is the softmax
     denominator for free.
  C) Per head: out_h[t, :] = attn_aug_h^T-slice.T @ Wo_aug_h (K=65).
     Wo_aug row 64 multiplies sumexp[t], so after the 1/sumexp[t]
     normalization it contributes a constant row: both b_o and the
     entire effect of b_v (= b_v_h @ W_o_h, since attn = attn0 +
     b_v*sumexp) are folded there on the host. Scaled by 1/sumexp[t]
     (per-partition scalar) during PSUM->SBUF combine.
     sumexp is transposed to [t, 1] with a tiny K=65 N=1 matmul against
     a unit vector (column 64 of the identity input).

The kernel is ScalarE-bound: 50.3M exp elements/core = ~390us of ACT
time; PE is ~330-415us busy.
"""
import sys
import os
import numpy as np

try:
    import jax
    jax.config.update("jax_compilation_cache_dir", "/tmp/jax_cache_mha")
    jax.config.update("jax_persistent_cache_min_compile_time_secs", 1.0)
except Exception:
    pass

if "/opt/trn_rl_repo" not in sys.path:
    sys.path.insert(0, "/opt/trn_rl_repo")

N_CORES = 8
B, T, D, H, DK = 2, 4096, 768, 12, 64
HPC = 3  # heads per core

_cache = {}


def _build_nc():
    import concourse.bass as bass  # noqa: F401
    import concourse.mybir as mybir
    import concourse.tile as tile
    from concourse import bacc

    f32 = mybir.dt.float32
    f32r = mybir.dt.float32r
    AF = mybir.ActivationFunctionType
    ALU = mybir.AluOpType

    NKC = T // 128   # 32 key chunks
    NQC = T // 512   # 8 query chunks

    nc = bacc.Bacc(None, target_bir_lowering=False)
    xbT = nc.dram_tensor("xbT", [D, T], f32r, kind="ExternalInput")
    ident_d = nc.dram_tensor("ident", [128, 128], f32r, kind="ExternalInput")
    onesc_d = nc.dram_tensor("onesc", [128, 32], f32r, kind="ExternalInput")
    # wqk: 3 stationary groups of 128 cols: [Q01 | K01 | Q2,K2]
    wqk = nc.dram_tensor("wqk", [D, 384], f32r, kind="ExternalInput")
    wv = nc.dram_tensor("wv", [D, 256], f32r, kind="ExternalInput")
    bpack = nc.dram_tensor("bpack", [128, 3], f32, kind="ExternalInput")
    woaug = nc.dram_tensor("woaug", [65, 3 * D], f32r, kind="ExternalInput")
    o = nc.dram_tensor("o", [T, D], f32, kind="ExternalOutput")

    with tile.TileContext(nc) as tc:
        with tc.tile_pool(name="pers", bufs=1) as pers, \
             tc.tile_pool(name="expp", bufs=4) as expp, \
             tc.tile_pool(name="aTp", bufs=2) as aTp, \
             tc.tile_pool(name="sbc", bufs=3) as sbc, \
             tc.tile_pool(name="outp", bufs=3) as outp, \
             tc.tile_pool(name="psB", bufs=2, space="PSUM") as psB:
            identr = pers.tile([128, 128], f32r, tag="ident")
            nc.sync.dma_start(out=identr, in_=ident_d[:, :])

            bias_t = pers.tile([128, 3], f32, tag="bias")
            nc.sync.dma_start(out=bias_t, in_=bpack[:, :])
            wo_t = pers.tile([65, 3 * D], f32r, tag="wo")
            nc.sync.dma_start(out=wo_t, in_=woaug[:, :])

            qTA = pers.tile([128, T], f32r, tag="qTA")
            kTA = pers.tile([128, T], f32r, tag="kTA")
            qTB = pers.tile([128, T], f32r, tag="qTB")
            kTB = pers.tile([128, T], f32r, tag="kTB")
            vaug = [pers.tile([128, NKC * 65], f32r, tag=f"vaug{h}",
                              name=f"vaug{h}") for h in range(HPC)]
            for h in range(HPC):
                ones_view = vaug[h].rearrange("p (k c) -> p k c", c=65)[:, :, 64]
                nc.sync.dma_start(out=ones_view, in_=onesc_d[:, :])

            # ---- attention pass plumbing (psB tiles; usable during A) ----
            def start_pass(kT, qT, qsA, qsB, hA, hB):
                atA = psB.tile([65, 512], f32, tag="at", name="atA")
                atB = psB.tile([65, 512], f32, tag="at", name="atB")
                return (kT, qT, qsA, qsB, hA, hB, atA, atB)

            def emit_ss(st, ss):
                kT, qT, qsA, qsB, hA, hB, atA, atB = st
                kcs = (2 * ss, 2 * ss + 1)
                scA = psB.tile([128, 1024], f32, tag="sc", name="scA")
                scB = psB.tile([128, 1024], f32, tag="sc", name="scB")
                for j, kc in enumerate(kcs):
                    cs = slice(j * 512, (j + 1) * 512)
                    nc.tensor.matmul(
                        scA[:, cs], kT[0:64, kc * 128:(kc + 1) * 128],
                        qT[0:64, qsA], start=True, stop=True,
                        tile_position=(0, 0), skip_group_check=True)
                    nc.tensor.matmul(
                        scB[:, cs], kT[64:128, kc * 128:(kc + 1) * 128],
                        qT[64:128, qsB], start=True, stop=True,
                        tile_position=(64, 0), skip_group_check=True)
                eA = expp.tile([128, 1024], f32r, tag="exp", name="eA")
                eB = expp.tile([128, 1024], f32r, tag="exp", name="eB")
                nc.scalar.activation(eA, scA, AF.Exp, scale=0.125)
                nc.scalar.activation(eB, scB, AF.Exp, scale=0.125)
                for j, kc in enumerate(kcs):
                    cs = slice(j * 512, (j + 1) * 512)
                    nc.tensor.matmul(
                        atA, vaug[hA][:, kc * 65:kc * 65 + 65], eA[:, cs],
                        start=(kc == 0), stop=(kc == NKC - 1),
                        skip_group_check=True)
                    nc.tensor.matmul(
                        atB, vaug[hB][:, kc * 65:kc * 65 + 65], eB[:, cs],
                        start=(kc == 0), stop=(kc == NKC - 1),
                        skip_group_check=True)

            def finish_pass(st, dstA, dstB):
                nc.vector.tensor_copy(dstA, st[6])
                nc.vector.tensor_copy(dstB, st[7])

            # first pair's heads-0/1 pass for q-chunk 0 is interleaved
            # into phase A below (superstep j consumes key chunks 2j,2j+1
            # which phase A's tcb=j//2 iteration produces).
            aT_p0a = [aTp.tile([65, 512], f32r, tag=f"aTa{h}",
                               name=f"aTa{h}") for h in range(HPC)]
            st0 = start_pass(kTA, qTA, slice(0, 512), slice(0, 512), 0, 1)

            # ============ Phase A: x^T, QKV projections, V_aug ============
            with tc.tile_pool(name="pA", bufs=1) as pA, \
                 tc.tile_pool(name="xTp", bufs=8) as xTp, \
                 tc.tile_pool(name="psA", bufs=2, space="PSUM") as psA:

                wqk_t = [pA.tile([128, 384], f32r, tag=f"wqk{dc}",
                                 name=f"wqk{dc}") for dc in range(6)]
                wv_t = [pA.tile([128, 256], f32r, tag=f"wv{dc}",
                                name=f"wv{dc}") for dc in range(6)]
                for dc in range(6):
                    nc.sync.dma_start(out=wqk_t[dc],
                                      in_=wqk[dc * 128:(dc + 1) * 128, :])
                    nc.sync.dma_start(out=wv_t[dc],
                                      in_=wv[dc * 128:(dc + 1) * 128, :])

                for tcb in range(NQC):
                    tcols = slice(tcb * 512, (tcb + 1) * 512)
                    xts = []
                    for dc in range(6):
                        xt = xTp.tile([128, 512], f32r, tag="xT")
                        nc.sync.dma_start(
                            out=xt,
                            in_=xbT[dc * 128:(dc + 1) * 128, tcols])
                        xts.append(xt)
                    # Q/K projections (transposed layout), stationary = weights
                    for g in range(3):
                        pj = psA.tile([128, 512], f32, tag="pv", name="pj")
                        for dc in range(6):
                            nc.tensor.matmul(
                                pj, wqk_t[dc][:, g * 128:(g + 1) * 128],
                                xts[dc], start=(dc == 0), stop=(dc == 5),
                                skip_group_check=True)
                        if g == 0:
                            nc.vector.tensor_scalar_add(
                                qTA[:, tcols], pj, bias_t[:, 0:1])
                        elif g == 1:
                            nc.vector.tensor_scalar_add(
                                kTA[:, tcols], pj, bias_t[:, 1:2])
                        else:
                            # rows 0:64 = Q2, rows 64:128 = K2 (lane-locked)
                            nc.vector.tensor_scalar_add(
                                qTB[0:64, tcols], pj[0:64, :],
                                bias_t[0:64, 2:3])
                            nc.vector.tensor_scalar_add(
                                kTB[64:128, tcols], pj[64:128, :],
                                bias_t[64:128, 2:3])
                            # duplicate halves (partition shift via DMA)
                            nc.sync.dma_start(out=qTB[64:128, tcols],
                                              in_=qTB[0:64, tcols])
                            nc.sync.dma_start(out=kTB[0:64, tcols],
                                              in_=kTB[64:128, tcols])
                    # V in natural [t, dk] layout: stationary = x^T blocks
                    for i in range(4):
                        kc = tcb * 4 + i
                        vp = psA.tile([128, 512], f32, tag="pv", name="vp")[:, 0:256]
                        for dc in range(6):
                            nc.tensor.matmul(
                                vp, xts[dc][:, i * 128:(i + 1) * 128],
                                wv_t[dc], start=(dc == 0), stop=(dc == 5),
                                skip_group_check=True)
                        for h in range(HPC):
                            nc.vector.tensor_copy(
                                vaug[h][:, kc * 65:kc * 65 + 64],
                                vp[:, h * 64:(h + 1) * 64])
                    emit_ss(st0, 2 * tcb)
                    emit_ss(st0, 2 * tcb + 1)

            # ============ Phases B + C interleaved per query chunk ============
            with tc.tile_pool(name="psC", bufs=2, space="PSUM") as psC:

                def packed_pass(kT, qT, qsA, qsB, hA, hB, dstA, dstB):
                    st = start_pass(kT, qT, qsA, qsB, hA, hB)
                    for ss in range(NKC // 2):
                        emit_ss(st, ss)
                    finish_pass(st, dstA, dstB)

                def phase_c(qc, aT):
                    se_t = sbc.tile([128, 12], f32, tag="se", name="se_t")
                    rc_t = sbc.tile([128, 12], f32, tag="rc", name="rc_t")
                    for i in range(4):
                        for h in range(HPC):
                            sp = psC.tile([128, 1], f32, tag="op",
                                          name="sump")
                            nc.tensor.matmul(
                                sp,
                                aT[h][:, i * 128:(i + 1) * 128].bitcast(f32),
                                identr[0:65, 64:65].bitcast(f32),
                                start=True, stop=True,
                                skip_group_check=True)
                            nc.vector.tensor_copy(
                                se_t[:, i * 3 + h:i * 3 + h + 1], sp)
                    nc.vector.reciprocal(rc_t, se_t)
                    for i in range(4):
                        ot = outp.tile([128, D], f32, tag="ot", name="ot")
                        lcs = slice(i * 128, (i + 1) * 128)
                        for half in range(2):
                            hc = slice(half * 384, half * 384 + 384)
                            ps_h = []
                            for h in range(HPC):
                                ph = psC.tile([128, 384], f32, tag="op",
                                              name=f"op{h}")
                                nc.tensor.matmul(
                                    ph, aT[h][:, lcs],
                                    wo_t[:, h * D + half * 384:
                                         h * D + half * 384 + 384],
                                    start=True, stop=True,
                                    skip_group_check=True)
                                ps_h.append(ph)
                            t0 = sbc.tile([128, 384], f32, tag="t0",
                                          name="t0")
                            nc.vector.tensor_scalar_mul(
                                t0, ps_h[0], rc_t[:, i * 3:i * 3 + 1])
                            t1 = sbc.tile([128, 384], f32, tag="t1",
                                          name="t1")
                            nc.vector.scalar_tensor_tensor(
                                t1, ps_h[1], rc_t[:, i * 3 + 1:i * 3 + 2], t0,
                                ALU.mult, ALU.add)
                            nc.vector.scalar_tensor_tensor(
                                ot[:, hc], ps_h[2],
                                rc_t[:, i * 3 + 2:i * 3 + 3], t1,
                                ALU.mult, ALU.add)
                        r0 = qc * 512 + i * 128
                        nc.sync.dma_start(out=o[r0:r0 + 128, :], in_=ot)

                # software-pipelined pairs: each pair's phase C is
                # emitted after the NEXT pair's passes, so its PE/DVE work
                # gap-fills under that pair's exp stream instead of
                # stalling it at the boundary.
                carry = None
                for qp in range(NQC // 2):
                    qa, qb = 2 * qp, 2 * qp + 1
                    qsa = slice(qa * 512, (qa + 1) * 512)
                    qsb = slice(qb * 512, (qb + 1) * 512)
                    if qp == 0:
                        aTa = aT_p0a
                        finish_pass(st0, aTa[0], aTa[1])
                    else:
                        aTa = [aTp.tile([65, 512], f32r, tag=f"aTa{h}",
                                        name=f"aTa{h}") for h in range(HPC)]
                        packed_pass(kTA, qTA, qsa, qsa, 0, 1, aTa[0], aTa[1])
                    aTb = [aTp.tile([65, 512], f32r, tag=f"aTb{h}",
                                    name=f"aTb{h}") for h in range(HPC)]
                    # head 2 packed across the two q chunks, before the qb
                    # pass so phase_c(qa) overlaps the qb exp stream
                    packed_pass(kTB, qTB, qsa, qsb, 2, 2, aTa[2], aTb[2])
                    packed_pass(kTA, qTA, qsb, qsb, 0, 1, aTb[0], aTb[1])
                    if carry is not None:
                        phase_c(carry[0], carry[1])
                        phase_c(carry[2], carry[3])
                    carry = (qa, aTa, qb, aTb)
                phase_c(carry[0], carry[1])
                phase_c(carry[2], carry[3])

    nc.finalize()
    return nc


def _get_nc():
    if "nc" not in _cache:
        _cache["nc"] = _build_nc()
    return _cache["nc"]


def _make_in_maps(x, W_q, b_q, W_k, b_k, W_v, b_v, W_o, b_o):
    in_maps = []
    for c in range(N_CORES):
        b = c // 4
        h0 = (c % 4) * HPC  # first global head on this core
        c0 = h0 * DK        # first column of this core's heads
        g0 = W_q[:, c0:c0 + 128]
        g1 = W_k[:, c0:c0 + 128]
        g2 = np.concatenate([W_q[:, c0 + 128:c0 + 192],
                             W_k[:, c0 + 128:c0 + 192]], axis=1)
        wqk = np.concatenate([g0, g1, g2], axis=1)

        bpack = np.zeros((128, 3), np.float32)
        bpack[:, 0] = b_q[c0:c0 + 128]
        bpack[:, 1] = b_k[c0:c0 + 128]
        bpack[0:64, 2] = b_q[c0 + 128:c0 + 192]
        bpack[64:128, 2] = b_k[c0 + 128:c0 + 192]

        woaug = np.zeros((65, 3 * D), np.float32)
        for j in range(HPC):
            wo_h = W_o[c0 + j * DK:c0 + (j + 1) * DK, :]
            woaug[0:64, j * D:(j + 1) * D] = wo_h
            # b_v's effect on the normalized output is the constant
            # b_v_h @ W_o_h per head (attn = attn0 + b_v*sumexp); ride
            # the sumexp row like b_o does.
            woaug[64, j * D:(j + 1) * D] = b_v[c0 + j * DK:c0 + (j + 1) * DK] @ wo_h
        if c % 4 == 0:
            woaug[64, 0:D] += b_o  # b_o folded once per batch

        in_maps.append({
            "xbT": np.ascontiguousarray(x[b].T),
            "ident": np.eye(128, dtype=np.float32),
            "onesc": np.ones((128, 32), np.float32),
            "wqk": np.ascontiguousarray(wqk),
            "wv": np.ascontiguousarray(np.concatenate(
                [W_v[:, c0:c0 + 192], np.zeros((D, 64), np.float32)], axis=1)),
            "bpack": bpack,
            "woaug": woaug,
        })
    return in_maps


def kernel(**inputs):
    from concourse.bass_utils import run_bass_kernel_spmd

    args = {k: np.asarray(v, dtype=np.float32) for k, v in inputs.items()}
    in_maps = _make_in_maps(
        args["x"], args["W_q"], args["b_q"], args["W_k"], args["b_k"],
        args["W_v"], args["b_v"], args["W_o"], args["b_o"])

    nc = _get_nc()
    trace = bool(int(os.environ.get("KBENCH_TRACE", "0")))
    res = run_bass_kernel_spmd(nc, in_maps, core_ids=list(range(N_CORES)),
                               trace=trace)
    _cache["last_result"] = res

    out = np.zeros((B, T, D), np.float32)
    for c in range(N_CORES):
        out[c // 4] += res.results[c]["o"]
    return out



# revision 4
# speedup vs baseline: 1.3736x; 1.3736x over previous
"""Multi-head attention (B=2, T=4096, D=768, H=12) as a Bass/Tile kernel
for 8 Trainium2 NeuronCores.

Sharding: cores 0-3 own batch 0, cores 4-7 own batch 1; each core owns 3
heads. Host folds all bias constants (b_o and the b_v @ W_o terms) into a
single per-batch row added after the cross-core partial-sum gather.

Per-core pipeline (all matmuls bf16 in / fp32 PSUM out):
  A) x^T arrives bf16. Q^T/K^T [64|128, T] come from matmuls with W
     stationary and x^T moving; the PSUM->SBUF conversion adds b_q/b_k
     and narrows to bf16. V is produced in natural [t, dk] layout (x^T
     blocks stationary, W_v moving) and written bf16 into per-key-chunk
     V_aug tiles [128, 3*65] whose per-head 65th column is 1.0.
  B) scores^T[k, q] = K^T-chunk.T @ Q^T (N=512 moving columns). exp is
     split across three engines: ACT computes true exp (scale fused,
     bf16 out); DVE and Pool compute a one-op Schraudolph exp2: the fp32
     affine s*AS + (1.5*2^23 + c) rounds to an integer whose low
     half-word IS the bf16 bit pattern of ~exp(s/8); the attention
     matmul reads it through a stride-2 bf16 view.
  C) attn[q, 65]_h accumulates exp-chunk.T @ V_aug over 32 key chunks
     (moving is the 65-wide V_aug, not the 512-wide query stream; column
     64 yields sumexp[q] per partition). After 1/sumexp normalization
     (Pool) the [q, 195] tile is DMA-transposed (XBAR) into two [128,128]
     stationary tiles and projected against W_o (N=768 moving); the
     normalized aug columns == 1 land on zeroed W_o rows.
"""
import sys
import os
import numpy as np

try:
    import jax
    jax.config.update("jax_compilation_cache_dir", "/tmp/jax_cache_mha")
    jax.config.update("jax_persistent_cache_min_compile_time_secs", 1.0)
except Exception:
    pass

if "/opt/trn_rl_repo" not in sys.path:
    sys.path.insert(0, "/opt/trn_rl_repo")

N_CORES = 8
B, T, D, H, DK = 2, 4096, 768, 12, 64
HPC = 3           # heads per core
NKC = T // 128    # 32 key chunks
NQB = T // 512    # 8 query blocks
LAG = 2           # attnV trails scores by this many key chunks
TAIL1_KC = 2      # normalize+transpose of prev block emitted at this kc
TAIL2_KC = 8      # O-projection of prev block emitted at this kc

# exp engine pattern over (kc*3+h) slots: A=ACT true exp, D=DVE, P=Pool
EXP_PATTERN = "AADAPADAADAPADAD"

# Schraudolph: low half-word of fp32(s*AS + BS) is the bf16 bit pattern of
# exp(s*0.125)*(1+eps), |eps| <= 3%. AS = 0.125*128/ln2. BS centers the
# piecewise-linear 2^f ratio (1+f)/2^f (max 1.0614) to equal ripple.
_AS = 0.125 * 128.0 / np.log(2.0)
_BS = 12582912.0 + 16256.0 - 128.0 * 0.5 * np.log2(
    (1 + 1 / np.log(2)) / np.exp2(1 / np.log(2) - 1))

_cache = {}


def _build_nc():
    import concourse.bass as bass  # noqa: F401
    import concourse.mybir as mybir
    import concourse.tile as tile
    from concourse import bacc

    f32 = mybir.dt.float32
    bf16 = mybir.dt.bfloat16
    AF = mybir.ActivationFunctionType
    ALU = mybir.AluOpType

    nc = bacc.Bacc(None, target_bir_lowering=False)
    xbT = nc.dram_tensor("xbT", [D, T], bf16, kind="ExternalInput")
    wqk = nc.dram_tensor("wqk", [D, 384], bf16, kind="ExternalInput")
    wv = nc.dram_tensor("wv", [D, 192], bf16, kind="ExternalInput")
    wo1 = nc.dram_tensor("wo1", [128, D], bf16, kind="ExternalInput")
    wo2 = nc.dram_tensor("wo2", [67, D], bf16, kind="ExternalInput")
    bpack = nc.dram_tensor("bpack", [128, 3], f32, kind="ExternalInput")
    o = nc.dram_tensor("o", [T, D], f32, kind="ExternalOutput")

    with tile.TileContext(nc) as tc:
        with tc.tile_pool(name="pers", bufs=1) as pers, \
             tc.tile_pool(name="expA", bufs=10) as expAp, \
             tc.tile_pool(name="expB", bufs=10) as expBp, \
             tc.tile_pool(name="attn", bufs=4) as attnp, \
             tc.tile_pool(name="accp", bufs=2, space="PSUM") as accp, \
             tc.tile_pool(name="shp", bufs=4, space="PSUM") as shp:

            # ---------------- persistent SBUF ----------------
            wqk_t = pers.tile([128, 6 * 384], bf16, tag="wqk")
            nc.sync.dma_start(
                out=wqk_t.rearrange("p (a c) -> p a c", a=6),
                in_=wqk[:, :].rearrange("(a p) c -> p a c", p=128))
            wv_t = pers.tile([128, 6 * 192], bf16, tag="wv")
            nc.sync.dma_start(
                out=wv_t.rearrange("p (a c) -> p a c", a=6),
                in_=wv[:, :].rearrange("(a p) c -> p a c", p=128))
            wo1_t = pers.tile([128, D], bf16, tag="wo1")
            nc.sync.dma_start(out=wo1_t, in_=wo1[:, :])
            wo2_t = pers.tile([67, D], bf16, tag="wo2")
            nc.sync.dma_start(out=wo2_t, in_=wo2[:, :])
            bias_t = pers.tile([128, 3], f32, tag="bias")
            nc.sync.dma_start(out=bias_t, in_=bpack[:, :])

            xt = [pers.tile([128, T], bf16, tag=f"xt{dc}", name=f"xt{dc}")
                  for dc in range(6)]
            for dc in range(6):
                nc.sync.dma_start(out=xt[dc],
                                  in_=xbT[dc * 128:(dc + 1) * 128, :])

            qTA = pers.tile([128, T], bf16, tag="qTA")
            kTA = pers.tile([128, T], bf16, tag="kTA")
            qTB = pers.tile([64, T], bf16, tag="qTB")
            kT2s = pers.tile([128, T], bf16, tag="kT2s")  # rows 64:128 used
            kTB = pers.tile([64, T], bf16, tag="kTB")

            # V_aug: per key chunk [128, 3*65] bf16, col 65h+64 = 1.0
            vaug = pers.tile([128, NKC * 195], bf16, tag="vaug")
            vaug4 = vaug.rearrange("p (k h c) -> p k h c", k=NKC, h=3)
            nc.gpsimd.memset(vaug4[:, :, :, 64], 1.0)

            exp_tiles = {}   # (b, kc, h) -> ("a"| "b", tile)
            tailst = {}      # (b, q4) -> (aT1, aT2)

            # ---------------- emit helpers ----------------
            def emit_scores_exp(b, kc):
                qs = slice(b * 512, (b + 1) * 512)
                ks = slice(kc * 128, (kc + 1) * 128)
                for h in range(HPC):
                    sc = shp.tile([128, 512], f32, tag="ps", name="sc")
                    if h == 0:
                        nc.tensor.matmul(sc, kTA[0:64, ks], qTA[0:64, qs],
                                         start=True, stop=True,
                                         skip_group_check=True)
                    elif h == 1:
                        nc.tensor.matmul(sc, kTA[64:128, ks], qTA[64:128, qs],
                                         start=True, stop=True,
                                         tile_position=(64, 0),
                                         skip_group_check=True)
                    else:
                        nc.tensor.matmul(sc, kTB[:, ks], qTB[:, qs],
                                         start=True, stop=True,
                                         skip_group_check=True)
                    eng = EXP_PATTERN[(kc * HPC + h) % len(EXP_PATTERN)]
                    if eng == "A":
                        e = expAp.tile([128, 512], bf16, tag="ea", name="ea")
                        nc.scalar.activation(e, sc, AF.Exp, scale=0.125)
                        exp_tiles[(b, kc, h)] = ("a", e)
                    else:
                        e = expBp.tile([128, 512], f32, tag="eb", name="eb")
                        engine = nc.vector if eng == "D" else nc.gpsimd
                        engine.tensor_scalar(e, sc, float(_AS), float(_BS),
                                             ALU.mult, ALU.add)
                        exp_tiles[(b, kc, h)] = ("b", e)

            def emit_attnv(b, kc, acc01, acc23):
                for h in range(HPC):
                    kind, e = exp_tiles.pop((b, kc, h))
                    if kind == "a":
                        full = e
                    else:
                        full = e.bitcast(bf16).rearrange(
                            "p (c x) -> p c x", x=2)[:, :, 0]
                    for q4 in range(4):
                        stat = full[:, q4 * 128:(q4 + 1) * 128]
                        acc = acc01 if q4 < 2 else acc23
                        off = (q4 & 1) * 256 + h * 65
                        nc.tensor.matmul(
                            acc[:, off:off + 65], stat,
                            vaug[:, kc * 195 + h * 65:kc * 195 + h * 65 + 65],
                            start=(kc == 0 and h == 0 and (q4 & 1) == 0),
                            stop=(kc == NKC - 1 and h == HPC - 1
                                  and (q4 & 1) == 1),
                            skip_group_check=True)

            def emit_tail1(b, acc01, acc23):
                """reciprocal + normalize (bf16) + XBAR transposes."""
                for q4 in range(4):
                    acc = acc01 if q4 < 2 else acc23
                    off = (q4 & 1) * 256
                    rc = attnp.tile([128, 4], f32, tag="rc", name="rc")
                    se = acc[:, off:off + 195].rearrange(
                        "p (c x) -> p c x", x=65)[:, :, 64]
                    nc.vector.reciprocal(rc[:, 0:3], se)
                    an = attnp.tile([128, 256], bf16, tag="an", name="an")
                    for h in range(HPC):
                        nc.gpsimd.tensor_scalar(
                            an[:, h * 65:h * 65 + 65],
                            acc[:, off + h * 65:off + h * 65 + 65],
                            rc[:, h:h + 1], None, ALU.mult)
                    nc.gpsimd.memset(an[:, 195:256], 0.0)
                    aT1 = attnp.tile([128, 128], bf16, tag="aT1", name="aT1")
                    aT2 = attnp.tile([128, 128], bf16, tag="aT2", name="aT2")
                    nc.sync.dma_start_transpose(aT1, an[:, 0:128])
                    nc.sync.dma_start_transpose(aT2, an[:, 128:256])
                    tailst[(b, q4)] = (aT1, aT2)

            def emit_tail2(b, opp):
                """O-projection + store for block b (after tail1)."""
                for q4 in range(4):
                    aT1, aT2 = tailst.pop((b, q4))
                    op1 = opp.tile([128, 512], f32, tag="op", name="op1")
                    op2 = opp.tile([128, 512], f32, tag="op", name="op2")
                    for half, op in ((0, op1), (1, op2)):
                        hc = slice(half * 384, half * 384 + 384)
                        nc.tensor.matmul(op[:, 0:384], aT1, wo1_t[:, hc],
                                         start=True, stop=False,
                                         skip_group_check=True)
                        nc.tensor.matmul(op[:, 0:384], aT2[0:67, :],
                                         wo2_t[:, hc], start=False, stop=True,
                                         skip_group_check=True)
                    ot = attnp.tile([128, D], f32, tag="ot", name="ot")
                    nc.gpsimd.tensor_copy(ot[:, 0:384], op1[:, 0:384])
                    nc.gpsimd.tensor_copy(ot[:, 384:768], op2[:, 0:384])
                    r0 = b * 512 + q4 * 128
                    nc.sync.dma_start(out=o[r0:r0 + 128, :], in_=ot)

            # ---------------- phase A (+ block-0 interleave) ----------------
            b0_acc01 = accp.tile([128, 512], f32, tag="acc", name="acc0")
            b0_acc23 = accp.tile([128, 512], f32, tag="acc", name="acc1")

            with tc.tile_pool(name="projp", bufs=2, space="PSUM") as projp:
                for t in range(8):
                    tcols = slice(t * 512, (t + 1) * 512)
                    for g in range(3):
                        pj = projp.tile([128, 512], f32, tag="pj", name="pj")
                        for dc in range(6):
                            nc.tensor.matmul(
                                pj, wqk_t[:, dc * 384 + g * 128:
                                          dc * 384 + (g + 1) * 128],
                                xt[dc][:, tcols], start=(dc == 0),
                                stop=(dc == 5), skip_group_check=True)
                        if g == 0:
                            nc.vector.tensor_scalar(
                                qTA[:, tcols], pj, bias_t[:, 0:1], None,
                                ALU.add)
                        elif g == 1:
                            nc.vector.tensor_scalar(
                                kTA[:, tcols], pj, bias_t[:, 1:2], None,
                                ALU.add)
                        else:
                            nc.vector.tensor_scalar(
                                qTB[:, tcols], pj[0:64, :],
                                bias_t[0:64, 2:3], None, ALU.add)
                            nc.vector.tensor_scalar(
                                kT2s[64:128, tcols], pj[64:128, :],
                                bias_t[64:128, 2:3], None, ALU.add)
                    if t % 2 == 1:
                        sh = slice((t - 1) * 512, (t + 1) * 512)
                        nc.sync.dma_start(out=kTB[:, sh],
                                          in_=kT2s[64:128, sh])
                    for i in range(4):
                        kc = t * 4 + i
                        vp = projp.tile([128, 512], f32, tag="pj", name="vp")
                        for dc in range(6):
                            nc.tensor.matmul(
                                vp[:, 0:192],
                                xt[dc][:, kc * 128:(kc + 1) * 128],
                                wv_t[:, dc * 192:(dc + 1) * 192],
                                start=(dc == 0), stop=(dc == 5),
                                skip_group_check=True)
                        nc.vector.tensor_copy(
                            vaug4[:, kc, :, 0:64],
                            vp[:, 0:192].rearrange("p (h c) -> p h c", h=3))
                    if t >= 1:
                        for kc in range(4 * (t - 1), 4 * t):
                            emit_scores_exp(0, kc)
                            if kc >= LAG:
                                emit_attnv(0, kc - LAG, b0_acc01, b0_acc23)

            # ---------------- blocks ----------------
            with tc.tile_pool(name="opp", bufs=2, space="PSUM") as opp:
                carry = None
                cur = (0, b0_acc01, b0_acc23)
                for b in range(NQB):
                    if b == 0:
                        for kc in range(28, 32):
                            emit_scores_exp(0, kc)
                            emit_attnv(0, kc - LAG, cur[1], cur[2])
                        for kc in range(NKC - LAG, NKC):
                            emit_attnv(0, kc, cur[1], cur[2])
                    else:
                        acc01 = accp.tile([128, 512], f32, tag="acc",
                                          name="acc0")
                        acc23 = accp.tile([128, 512], f32, tag="acc",
                                          name="acc1")
                        cur = (b, acc01, acc23)
                        for kc in range(NKC):
                            emit_scores_exp(b, kc)
                            if kc >= LAG:
                                emit_attnv(b, kc - LAG, acc01, acc23)
                            if kc == TAIL1_KC and carry is not None:
                                emit_tail1(carry[0], carry[1], carry[2])
                            if kc == TAIL2_KC and carry is not None:
                                emit_tail2(carry[0], opp)
                        for kc in range(NKC - LAG, NKC):
                            emit_attnv(b, kc, acc01, acc23)
                    carry = cur
                emit_tail1(carry[0], carry[1], carry[2])
                emit_tail2(carry[0], opp)

    nc.finalize()
    return nc


def _get_nc():
    if "nc" not in _cache:
        _cache["nc"] = _build_nc()
    return _cache["nc"]


def _make_in_maps(x, W_q, b_q, W_k, b_k, W_v, b_v, W_o, b_o):
    import ml_dtypes
    bf = ml_dtypes.bfloat16
    in_maps = []
    for c in range(N_CORES):
        b = c // 4
        h0 = (c % 4) * HPC
        c0 = h0 * DK

        g0 = W_q[:, c0:c0 + 128]
        g1 = W_k[:, c0:c0 + 128]
        g2 = np.concatenate([W_q[:, c0 + 128:c0 + 192],
                             W_k[:, c0 + 128:c0 + 192]], axis=1)
        wqk_m = np.concatenate([g0, g1, g2], axis=1)

        bpack = np.zeros((128, 3), np.float32)
        bpack[:, 0] = b_q[c0:c0 + 128]
        bpack[:, 1] = b_k[c0:c0 + 128]
        bpack[0:64, 2] = b_q[c0 + 128:c0 + 192]
        bpack[64:128, 2] = b_k[c0 + 128:c0 + 192]

        # woaug [195, 768]: per head rows 0..63 = W_o rows; row 64 = 0
        woaug = np.zeros((195, D), np.float32)
        for j in range(HPC):
            woaug[j * 65:j * 65 + 64, :] = \
                W_o[c0 + j * DK:c0 + (j + 1) * DK, :]

        in_maps.append({
            "xbT": np.ascontiguousarray(x[b].T).astype(bf),
            "wqk": np.ascontiguousarray(wqk_m).astype(bf),
            "wv": np.ascontiguousarray(W_v[:, c0:c0 + 192]).astype(bf),
            "wo1": np.ascontiguousarray(woaug[0:128, :]).astype(bf),
            "wo2": np.ascontiguousarray(woaug[128:195, :]).astype(bf),
            "bpack": bpack,
        })
    return in_maps


def kernel(**inputs):
    from concourse.bass_utils import run_bass_kernel_spmd

    args = {k: np.asarray(v, dtype=np.float32) for k, v in inputs.items()}
    in_maps = _make_in_maps(
        args["x"], args["W_q"], args["b_q"], args["W_k"], args["b_k"],
        args["W_v"], args["b_v"], args["W_o"], args["b_o"])

    nc = _get_nc()
    trace = bool(int(os.environ.get("KBENCH_TRACE", "0")))
    res = run_bass_kernel_spmd(nc, in_maps, core_ids=list(range(N_CORES)),
                               trace=trace)
    _cache["last_result"] = res

    out = np.zeros((B, T, D), np.float32)
    for c in range(N_CORES):
        out[c // 4] += res.results[c]["o"]
    # bias constants folded on host: b_o plus every head's b_v @ W_o
    bias_row = args["b_o"] + args["b_v"] @ args["W_o"]
    out += bias_row[None, None, :]
    return out
